# revision 24
# baseline (speedup 1.0000x reference)
"""Trainium2 Bass kernel for ConvEncoderND (SetConv encoder + pointwise MLP).

Math (per batch element b, one NeuronCore each):
    q[i,o]   = || x_grid[o] - x_context[i] ||^2 + EPSQ       (n_in x n_out)
    D[i,o]   = sqrt(q)
    E[i,o]   = exp(a * D)            a = -0.5/exp(sigma)^2   (equal-sigma path)
    dens[o]  = sum_i E ;  conv[o] = sum_i y_i * E
    out[k,o] = sigmoid(W[k,0]*dens + W[k,1]*conv/(dens+1e-8) + b[k])

Engine assignment (equal-sigma fast path), o processed in 4 quarters of 1024:
  PE   : q via rank-4 f32r matmul; [dens;conv] reduce (bf16 E); stage-3 (bf16)
  ACT  : sqrt for chunks 0-1 (PSUM->SBUF bf16); final Sigmoid (1 table switch)
  Pool : sqrt for chunk 2 + most of chunk 3 (tensor_scalar pow 0.5)
  DVE  : exp as (e^a)^D = tensor_tensor(cbase, D, pow) in bf16 (2x mode);
         normalization in [128,8] layout: rc = (dens+1e-8)^-1 in one op
  DMA  : acc rows evacuated from PSUM by reshape DMAs [1,1024]<->[128,8]

PSUM: psq 2 x [128,1024] (4 banks) + pacc 2 x 2-bank slots for acc [2,1024]
and z [64,1024].
"""

import numpy as np
import ml_dtypes

import concourse.bass as bass
import concourse.tile as tile
from concourse import bacc, mybir
from concourse.bass_utils import run_bass_kernel_spmd
from concourse.tile_rust import add_dep_helper

AF = mybir.ActivationFunctionType
ALU = mybir.AluOpType
F32 = mybir.dt.float32
F32R = mybir.dt.float32r
BF16 = mybir.dt.bfloat16

B = 8
N_IN = 512
N_OUT = 4096
C_OUT = 64
IC = N_IN // 128      # 4 chunks of 128 context points (partition dim)
NQ = 4                # o-quarters
QW = N_OUT // NQ      # 1024
EPSQ = 5e-7           # folded into |xc|^2 so sqrt never sees a negative
SQRT_BIAS = 2e-5      # v2 margin: absorbs split-bf16 matmul rounding in q


def _build_program_v2(a0: float, kept: tuple):
    """Equal-sigma fast path. a0 = -0.5/scale^2 baked in as e^{a0} immediate.

    kept: sorted tuple of (t, c) pairs to process; context is sorted by x on
    the host into 4 bands (c) and o-quarters (t) are x-strips, so far pairs
    have exp(a0 * gap) ~ 0 and are skipped entirely.
    """
    nc = bacc.Bacc(
        "TRN2",
        target_bir_lowering=False,
        debug=False,
        num_devices=B,
    )

    A_d = nc.dram_tensor("A", [12, N_IN], BF16, kind="ExternalInput")
    R_d = nc.dram_tensor("R", [12, N_OUT], BF16, kind="ExternalInput")
    Y2_d = nc.dram_tensor("Y2", [IC, 128, 2], BF16, kind="ExternalInput")
    WB_d = nc.dram_tensor("WB", [3, C_OUT], BF16, kind="ExternalInput")
    OUT_d = nc.dram_tensor("OUT", [C_OUT, N_OUT], F32, kind="ExternalOutput")

    cval = float(np.exp(a0))

    with tile.TileContext(nc) as tc:
        with (
            tc.tile_pool(name="const", bufs=1) as const,
            tc.tile_pool(name="dring", bufs=3) as dring,
            tc.tile_pool(name="ering", bufs=3) as ering,
            tc.tile_pool(name="psq", bufs=2, space=bass.MemorySpace.PSUM) as psq,
            tc.tile_pool(name="pacc", bufs=2, space=bass.MemorySpace.PSUM) as pacc,
        ):
            Asb = const.tile([12, N_IN], BF16)
            Rsb = const.tile([12, N_OUT], BF16)
            y2sb = const.tile([128, 2 * IC], BF16)
            wbsb = const.tile([3, C_OUT], BF16)
            cbase = const.tile([128, QW], BF16)
            v3 = const.tile([3, N_OUT], BF16)
            dn = const.tile([128, 8 * NQ], BF16)
            cv = const.tile([128, 8 * NQ], BF16)
            rc = const.tile([128, 8 * NQ], BF16)
            cvn = const.tile([128, 8 * NQ], BF16)
            tout = const.tile([C_OUT, N_OUT], F32)

            nc.sync.dma_start(out=Asb[:], in_=A_d[:])
            nc.sync.dma_start(out=Rsb[:], in_=R_d[:])
            for c in range(IC):
                nc.sync.dma_start(out=y2sb[:, 2 * c : 2 * c + 2], in_=Y2_d[c])
            nc.sync.dma_start(out=wbsb[:], in_=WB_d[:])
            nc.vector.memset(cbase[:], cval)
            ones_sb = const.tile([1, N_OUT], BF16)
            nc.vector.memset(ones_sb[:], 1.0)
            nc.sync.dma_start(out=v3[2:3, :], in_=ones_sb[:])

            act_sqrts = []
            sig_insts = []

            # exp goes to Pool except a few tiles on ACT when kept is large
            n_kept = len(kept)
            n_exp_act = max(0, int(round((0.48 * n_kept - 5.4) / 2.56)))
            exp_act = set(list(kept)[:n_exp_act])

            # pass 1: per-(t, c) pipeline PE q -> ACT sqrt -> Pool exp -> PE
            # reduce. All ACT sqrts are emitted before any sigmoid so the ACT
            # queue order matches the single table switch.
            accs = []
            for t in range(NQ):
                chunks = [c for c in range(IC) if (t, c) in kept]
                acc = pacc.tile([2, QW], F32, name=f"acc{t}", tag="pacc")
                accs.append(acc)
                for c in chunks:
                    q = psq.tile([128, QW], F32, name=f"q{t}{c}", tag="psq")
                    for j in range(2):
                        nc.tensor.matmul(
                            q[:, j * 512 : (j + 1) * 512],
                            Asb[:, c * 128 : (c + 1) * 128],
                            Rsb[:, t * QW + j * 512 : t * QW + (j + 1) * 512],
                            start=True,
                            stop=True,
                        )
                    D = dring.tile([128, QW], BF16, name=f"D{t}{c}", tag="dring")
                    s = nc.scalar.activation(D[:], q[:], AF.Sqrt)
                    act_sqrts.append(s)
                    E = ering.tile([128, QW], BF16, name=f"E{t}{c}", tag="ering")
                    nc.gpsimd.tensor_tensor(E[:], cbase[:], D[:], ALU.pow)
                    for j in range(2):
                        nc.tensor.matmul(
                            acc[:, j * 512 : (j + 1) * 512],
                            y2sb[:, 2 * c : 2 * c + 2],
                            E[:, j * 512 : (j + 1) * 512],
                            start=(c == chunks[0]),
                            stop=(c == chunks[-1]),
                        )

            # pass 2: per-quarter tails. DVE evacuates acc into the v3 rows,
            # reshape DMAs give [128, 8] tiles for full-lane DVE math, cvn
            # DMAs back into v3 row 1, then stage-3 + sigmoid + output.
            for t in range(NQ):
                tsl = slice(t * QW, (t + 1) * QW)
                fsl = slice(t * 8, (t + 1) * 8)
                acc = accs[t]
                nc.vector.tensor_copy(v3[0:2, tsl], acc[0:2, :])
                nc.sync.dma_start(out=dn[:, fsl], in_=v3[0:1, tsl])
                nc.sync.dma_start(out=cv[:, fsl], in_=v3[1:2, tsl])
                with nc.allow_low_precision(reason="bf16 norm, tol 2e-2"):
                    nc.vector.tensor_scalar_add(rc[:, fsl], dn[:, fsl], 1e-8)
                    nc.vector.reciprocal(rc[:, fsl], rc[:, fsl])
                    nc.vector.tensor_tensor(
                        cvn[:, fsl], cv[:, fsl], rc[:, fsl], ALU.mult
                    )
                nc.sync.dma_start(out=v3[1:2, tsl], in_=cvn[:, fsl])

                z = pacc.tile([C_OUT, QW], F32, name=f"z{t}", tag="pacc")
                for j in range(2):
                    nc.tensor.matmul(
                        z[:, j * 512 : (j + 1) * 512],
                        wbsb[:],
                        v3[:, t * QW + j * 512 : t * QW + (j + 1) * 512],
                        start=True,
                        stop=True,
                    )
                sg = nc.scalar.activation(tout[:, tsl], z[:], AF.Sigmoid)
                sig_insts.append(sg)
                nc.sync.dma_start(out=OUT_d[:, tsl], in_=tout[:, tsl])

            # single ACT table switch: all sqrts before any sigmoid
            for sg in sig_insts:
                for s in act_sqrts:
                    add_dep_helper(sg.ins, s.ins, False, "act table phase order")

    nc.compile()
    return nc


def _build_program_v1(a0: float, a1: float, equal_sigma: bool, mm_dtype: str):
    """Generic fallback (any sigma): original slower decomposition."""
    nc = bacc.Bacc(
        "TRN2",
        target_bir_lowering=False,
        debug=False,
        num_devices=B,
    )

    A_d = nc.dram_tensor("A", [4, N_IN], F32, kind="ExternalInput")
    R_d = nc.dram_tensor("R", [4, N_OUT], F32, kind="ExternalInput")
    Y2_d = nc.dram_tensor("Y2", [IC, 128, 6], BF16, kind="ExternalInput")
    WB_d = nc.dram_tensor("WB", [3, C_OUT], F32, kind="ExternalInput")
    OUT_d = nc.dram_tensor("OUT", [C_OUT, N_OUT], F32, kind="ExternalOutput")

    n_e = 1 if equal_sigma else 2
    HW_ = N_OUT // 2

    with tile.TileContext(nc) as tc:
        with (
            tc.tile_pool(name="const", bufs=1) as const,
            tc.tile_pool(name="dbuf", bufs=1) as dbuf,
            tc.tile_pool(name="ebuf", bufs=1) as ebuf,
            tc.tile_pool(name="psq", bufs=2, space=bass.MemorySpace.PSUM) as psq,
            tc.tile_pool(name="pst", bufs=1, space=bass.MemorySpace.PSUM) as pst,
        ):
            Asb = const.tile([4, N_IN], F32)
            Rsb = const.tile([4, N_OUT], F32)
            y2sb = const.tile([128, 6 * IC], BF16)
            wbsb = const.tile([3, C_OUT], F32)
            v3 = const.tile([3, N_OUT], F32)
            dn = const.tile([128, N_OUT // 128], F32)
            cv = const.tile([128, N_OUT // 128], F32)
            rc = const.tile([128, N_OUT // 128], F32)
            cvn = const.tile([128, N_OUT // 128], F32)
            tout = const.tile([C_OUT, N_OUT], F32)

            nc.sync.dma_start(out=Asb[:], in_=A_d[:])
            nc.sync.dma_start(out=Rsb[:], in_=R_d[:])
            for c in range(IC):
                nc.sync.dma_start(out=y2sb[:, 6 * c : 6 * c + 6], in_=Y2_d[c])
            nc.sync.dma_start(out=wbsb[:], in_=WB_d[:])
            ones_sb = const.tile([1, N_OUT], F32)
            nc.vector.memset(ones_sb[:], 1.0)
            nc.sync.dma_start(out=v3[2:3, :], in_=ones_sb[:])

            A_mm = Asb[:]
            R_mm = Rsb[:]

            D = dbuf.tile([128, IC * N_OUT], F32)
            Es = [
                ebuf.tile([128, IC * N_OUT], BF16, name=f"E{e}", tag=f"E{e}")
                for e in range(n_e)
            ]

            sqrt_insts = []
            QW1 = 1024
            for c in range(IC):
                for h in range(N_OUT // QW1):
                    q = psq.tile([128, QW1], F32, name=f"q{c}{h}", tag="psq")
                    for j4 in range(QW1 // 512):
                        o0 = h * QW1 + j4 * 512
                        nc.tensor.matmul(
                            q[:, j4 * 512 : (j4 + 1) * 512],
                            A_mm[:, c * 128 : (c + 1) * 128],
                            R_mm[:, o0 : o0 + 512],
                            start=True,
                            stop=True,
                        )
                    d_sl = D[:, c * N_OUT + h * QW1 : c * N_OUT + (h + 1) * QW1]
                    sqrt_insts.append(nc.scalar.activation(d_sl, q[:], AF.Sqrt))

            exp_insts = []
            scales = [a0] if equal_sigma else [a0, a1]
            for e, a in enumerate(scales):
                for c in range(IC):
                    d_sl = D[:, c * N_OUT : (c + 1) * N_OUT]
                    e_sl = Es[e][:, c * N_OUT : (c + 1) * N_OUT]
                    exp_insts.append(
                        nc.scalar.activation(e_sl, d_sl, AF.Exp, 0.0, a)
                    )
            for s in sqrt_insts:
                for x in exp_insts:
                    add_dep_helper(x.ins, s.ins, False, "act table phase order")

            for h in range(2):
                osl = slice(h * HW_, (h + 1) * HW_)
                fsl = slice(h * 16, (h + 1) * 16)

                acc = pst.tile([2, HW_], F32, name=f"acc{h}", tag="pst")
                if equal_sigma:
                    for c in range(IC):
                        for j in range(4):
                            nc.tensor.matmul(
                                acc[:, j * 512 : (j + 1) * 512],
                                y2sb[:, 6 * c : 6 * c + 2],
                                Es[0][
                                    :,
                                    c * N_OUT + h * HW_ + j * 512 :
                                    c * N_OUT + h * HW_ + (j + 1) * 512,
                                ],
                                start=(c == 0),
                                stop=(c == IC - 1),
                            )
                else:
                    for row in range(2):
                        for c in range(IC):
                            for j in range(4):
                                nc.tensor.matmul(
                                    acc[:, j * 512 : (j + 1) * 512],
                                    y2sb[:, 6 * c + 2 + 2 * row : 6 * c + 4 + 2 * row],
                                    Es[row][
                                        :,
                                        c * N_OUT + h * HW_ + j * 512 :
                                        c * N_OUT + h * HW_ + (j + 1) * 512,
                                    ],
                                    start=(row == 0 and c == 0),
                                    stop=(row == 1 and c == IC - 1),
                                )

                nc.vector.tensor_copy(v3[0:2, osl], acc[0:2, :])
                nc.sync.dma_start(out=dn[:, fsl], in_=v3[0:1, osl])
                nc.sync.dma_start(out=cv[:, fsl], in_=v3[1:2, osl])
                nc.vector.tensor_scalar_add(rc[:, fsl], dn[:, fsl], 1e-8)
                nc.vector.reciprocal(rc[:, fsl], rc[:, fsl])
                nc.vector.tensor_tensor(
                    cvn[:, fsl], cv[:, fsl], rc[:, fsl], ALU.mult
                )
                nc.sync.dma_start(out=v3[1:2, osl], in_=cvn[:, fsl])

                z = pst.tile([C_OUT, HW_], F32, name=f"z{h}", tag="pst")
                for j in range(4):
                    nc.tensor.matmul(
                        z[:, j * 512 : (j + 1) * 512],
                        wbsb[:],
                        v3[:, h * HW_ + j * 512 : h * HW_ + (j + 1) * 512],
                        start=True,
                        stop=True,
                    )
                th = nc.scalar.activation(tout[:, osl], z[:], AF.Tanh, 0.0, 0.5)
                for s in sqrt_insts:
                    add_dep_helper(th.ins, s.ins, False, "act table phase order")
                nc.vector.tensor_scalar(
                    tout[:, osl], tout[:, osl], 0.5, 0.5, ALU.mult, ALU.add
                )
                nc.sync.dma_start(out=OUT_d[:, osl], in_=tout[:, osl])

    nc.compile()
    return nc


def _prep_inputs(x_context, y_context, x_grid, sigma, W, b):
    """Host-side prep: per-core augmented tensors (all O(n) work)."""
    scales = np.exp(sigma.astype(np.float64))
    a = (-0.5 / scales**2).astype(np.float64)
    a0, a1 = float(a[0]), float(a[1])
    equal_sigma = abs(a0 - a1) <= 1e-9 * max(abs(a0), abs(a1))
    # v2 path needs e^{a0} representable in bf16
    use_v2 = equal_sigma and np.exp(a0) > 1.5e-38

    # v2 sparsity: context sorted into 4 x-bands, o-quarters are x-strips of
    # the ij-raveled grid; (t, c) pairs whose x-gap makes exp(a0 * gap)
    # negligible are skipped. kept is the union over batch elements.
    kept = set()
    orders = []
    if use_v2:
        thr = 15.3 / max(1e-9, -a0)  # 128*exp(a0*gap) < ~3e-5
        for bi in range(B):
            order = np.argsort(np.asarray(x_context[bi, :, 0]), kind="stable")
            orders.append(order)
            xs = np.asarray(x_context[bi, :, 0])[order]
            gx = np.asarray(x_grid[bi, :, 0])
            for t in range(NQ):
                # exact x-range of the grid points in o-quarter t
                strip = gx[t * QW : (t + 1) * QW]
                slo, shi = float(strip.min()), float(strip.max())
                for c in range(IC):
                    blo = float(xs[c * 128])
                    bhi = float(xs[c * 128 + 127])
                    gap = max(0.0, blo - shi, slo - bhi)
                    if gap < thr:
                        kept.add((t, c))
        for t in range(NQ):  # safety: never leave a quarter empty
            if not any((t, c) in kept for c in range(IC)):
                kept.add((t, min(IC - 1, t)))
    kept = tuple(sorted(kept))

    in_maps = []
    for bi in range(B):
        if use_v2:
            o = orders[bi]
            xc = np.asarray(x_context[bi])[o].astype(np.float32)  # (512, 2)
            yc = np.asarray(y_context[bi, :, 0])[o].astype(np.float32)
        else:
            xc = x_context[bi].astype(np.float32)
            yc = y_context[bi, :, 0].astype(np.float32)
        xg = x_grid[bi].astype(np.float32)     # (4096, 2)

        epsq = SQRT_BIAS if use_v2 else EPSQ
        cn = (xc[:, 0] ** 2 + xc[:, 1] ** 2 + epsq).astype(np.float32)
        gn = (xg[:, 0] ** 2 + xg[:, 1] ** 2).astype(np.float32)
        A = np.stack(
            [-2.0 * xc[:, 0], -2.0 * xc[:, 1], np.ones(N_IN, np.float32), cn]
        ).astype(np.float32)
        R = np.stack(
            [xg[:, 0], xg[:, 1], gn, np.ones(N_OUT, np.float32)]
        ).astype(np.float32)
        if use_v2:
            # two-term bf16 split: q = ah.rh + ah.rl + al.rh (al.rl dropped)
            ah = A.astype(ml_dtypes.bfloat16)
            al = (A - ah.astype(np.float32)).astype(ml_dtypes.bfloat16)
            rh = R.astype(ml_dtypes.bfloat16)
            rl = (R - rh.astype(np.float32)).astype(ml_dtypes.bfloat16)
            A = np.concatenate([ah, ah, al], axis=0)    # (12, N_IN) bf16
            R = np.concatenate([rh, rl, rh], axis=0)    # (12, N_OUT) bf16
        ones = np.ones(N_IN, np.float32)
        if use_v2:
            Y2 = np.stack([ones, yc], axis=-1)
            Y2 = Y2.reshape(IC, 128, 2).astype(ml_dtypes.bfloat16)
            WB = np.stack([W[:, 0], W[:, 1], b]).astype(ml_dtypes.bfloat16)
        else:
            zero = np.zeros(N_IN, np.float32)
            Y2 = np.stack([ones, yc, ones, zero, zero, yc], axis=-1)
            Y2 = Y2.reshape(IC, 128, 6).astype(ml_dtypes.bfloat16)
            WB = np.stack([W[:, 0], W[:, 1], b]).astype(np.float32)
        in_maps.append({"A": A, "R": R, "Y2": Y2, "WB": WB})
    return in_maps, a0, a1, equal_sigma, use_v2, kept


_PROGRAM_CACHE = {}


def run_device(inputs, mm_dtype="f32", trace=False):
    """Run the bass kernel; returns (output (B,64,64,64) f32, results)."""
    in_maps, a0, a1, equal_sigma, use_v2, kept = _prep_inputs(**inputs)
    key = (round(a0, 12), round(a1, 12), equal_sigma, use_v2, kept, mm_dtype)
    if key not in _PROGRAM_CACHE:
        if use_v2:
            _PROGRAM_CACHE[key] = _build_program_v2(a0, kept)
        else:
            _PROGRAM_CACHE[key] = _build_program_v1(a0, a1, equal_sigma, "f32")
    nc = _PROGRAM_CACHE[key]
    res = run_bass_kernel_spmd(nc, in_maps, core_ids=list(range(B)), trace=trace)
    out = np.stack([res.results[i]["OUT"] for i in range(B)])
    out = out.reshape(B, C_OUT, 64, 64).astype(np.float32)
    return out, res


def kernel(**inputs) -> np.ndarray:
    out, _ = run_device(inputs)
    return out
